# revision 49
# baseline (speedup 1.0000x reference)
"""Trainium2 Bass kernel for nn_Branch_2_36386962932308.

Network (per batch, feature-major planes [channels, L=h*w=4096]):
  stage1: Mamba(d=128, di=128, n=2, r=8, conv4) -> LN
  linear: 128->256 + SiLU   (stage-1 LN affine folded into the linear weight)
  stage2: Mamba(d=256, di=256, n=2, r=16, conv4) -> LN (affine applied on device)

Sharding: data-parallel over batch, one batch element per NeuronCore (8 cores).

Key restructurings:
  - Input x[b] is already the feature-major plane [c, h*w]; output likewise.
    No global transposes anywhere.
  - The causal depthwise conv (4 taps) is folded into in_proj: 4 shifted
    matmuls with host-precomputed weights (cw_k * win_x) accumulated in PSUM.
    3 leading zero columns on the input plane provide causal padding; a tiny
    correction fixes the folded input-bias on the first 3 columns.
  - The SSM scan uses the native DVE first-order recurrence
    tensor_tensor_scan (state = dA*state + dBu along the free dim), one
    instruction per [128, CH] chunk, chained across chunks via
    initial=prev[:, -1:].
  - B and C rows (per-timestep, shared across channels) are replicated across
    partitions with selection-matrix matmuls on the PE (rep = sel_j.T @ xdbl).
  - out_proj emits time-major [T=128, dout+1] tiles (extra weight column =
    rowmean, giving the per-timestep LN mean for free); variance comes from
    native bn_stats/bn_aggr on the DVE (the custom TensorTensorReduce ISA op
    crashes this runtime); normalize runs at 4x on bf16 SBUF tiles.
  - ACT table-set discipline: one SiLU phase then one Ln/Exp phase per stage
    (SPAN=L), softplus(x) = Ln(Exp(x)+1), rstd = Exp(-0.5*Ln(var+eps)).
    Stage-2's softplus/dA chain runs as single [128, 2*CH] ops (bdt and A0
    are per-half-constant); dA1 = dA0^2 on the Pool engine (A1 == 2*A0).
  - Both LN outputs leave time-major as bf16 via SBUF->SBUF DMA-transpose;
    the stage-2 LN affine runs per chunk on the Pool engine and outputs
    stream out via Pool-issued (SWDGE) DMAs, keeping HWDGE free.
  - Weights arrive packed in three blob DMAs (f32 stage-1, f32 misc, bf16
    projections); x is split into four DMAs so chunk-0 compute starts ~2us
    after launch.  Engine placement of movable ops is tuned via ENG.

Self-contained: hardcodes all shapes; needs only concourse + numpy at runtime.
"""

import os
from contextlib import ExitStack

import numpy as np

import concourse.bass as bass
import concourse.bacc as bacc
import concourse.mybir as mybir
import concourse.tile as tile
from concourse.bass_utils import run_bass_kernel_spmd

F32 = mybir.dt.float32
BF16 = mybir.dt.bfloat16
AF = mybir.ActivationFunctionType
ALU = mybir.AluOpType

NCORES = 8
LN_EPS = 1e-5
CH = 512          # pipeline column chunk (one PSUM bank at fp32)
SUB = 128         # out_proj / LN subchunk (time-major tile height)
SPAN = 4096       # ACT table-set phase width

last_exec_time_ns = None

if os.environ.get("KB_NOPOOL"):
    pass  # applied below after ENG is defined


def _patch_act_tables():
    """Make natural_log_exp_and_others the only table set containing Exp and
    Ln, so bacc's table-load placement keeps one set resident through the
    whole post-SiLU phase instead of swapping between exp_and_others and
    natural_log on every Exp<->Ln transition (~2.7us per swap). Set ids and
    ordering are preserved; only membership is filtered."""
    import functools
    import concourse.hw_specs as hw_specs
    if getattr(hw_specs.get_activation_tables, "_lnexp_patched", False):
        return
    orig = hw_specs.get_activation_tables

    @functools.cache
    def patched(arch):
        tables = {k: set(v) for k, v in orig(arch).items()}
        for name, fns in tables.items():
            if name != 'natural_log_exp_and_others':
                fns.discard(AF.Exp)
                fns.discard(AF.Ln)
        return tables

    patched._lnexp_patched = True
    hw_specs.get_activation_tables = patched
    bacc.get_activation_tables = patched


_patch_act_tables()


# ----------------------------------------------------------------------------
# host-side weight preparation
# ----------------------------------------------------------------------------

def _prep_stage(p, d, di, r):
    win = np.asarray(p['win'], np.float32)
    b_in = np.asarray(p['bin'], np.float32)
    cw = np.asarray(p['cw'], np.float32)        # [di, 1, 4]
    cb = np.asarray(p['cb'], np.float32)
    wx = np.asarray(p['wx'], np.float32)        # [r+4, di]
    wdt = np.asarray(p['wdt'], np.float32)      # [di, r]
    bdt = np.asarray(p['bdt'], np.float32)
    alog = np.asarray(p['alog'], np.float32)    # [di, 2]
    dd = np.asarray(p['dd'], np.float32)
    wout = np.asarray(p['wout'], np.float32)    # [dout, di]

    winx, winz = win[:di], win[di:]
    w_k = np.stack([np.ascontiguousarray((cw[:, 0, k:k + 1] * winx).T)
                    for k in range(4)])          # [4, d, di]
    wz = np.ascontiguousarray(winz.T)            # [d, di]
    wxT = np.ascontiguousarray(wx.T)             # [di, r+4]
    wdtT = np.ascontiguousarray(wdt.T)           # [r, di]
    woutT = np.ascontiguousarray(wout.T)         # [di, dout]

    S = cw[:, 0, :].sum(1)
    silu_bias = cb + S * b_in[:di]
    bz = b_in[di:]
    A = -np.exp(alog)                            # [di, 2] (negative)
    corr = np.stack([-(cw[:, 0, :3 - t].sum(1)) * b_in[:di] for t in range(3)], 1)
    cols = [silu_bias, bz, bdt, A[:, 0], A[:, 1], dd,
            corr[:, 0], corr[:, 1], corr[:, 2]]
    sel = np.zeros((4, r + 4, 128), np.float32)
    for j in range(4):
        sel[j, r + j, :] = 1.0
    return w_k, wz, wxT, wdtT, woutT, np.stack(cols, 1).astype(np.float32), sel


# f32 blob layouts: name -> (rows, cols).  Two blobs so stage-1 weights
# (needed first) arrive in a small early DMA; everything else follows.
_F32A_LAYOUT = [
    ('w1k0', 128, 128), ('w1k1', 128, 128), ('w1k2', 128, 128),
    ('w1k3', 128, 128), ('w1z', 128, 128),
    ('cols1', 128, 9), ('eps', 128, 1),
]
_F32B_LAYOUT = [
    ('linb', 128, 2),
    ('cols2_0', 128, 11), ('cols2_1', 128, 11),
]
# wout blocks carry an extra trailing column = rowsum(wout)/dout so the
# out_proj matmul emits the per-timestep mean for free.  All projection
# weights consumed against bf16 activations live here too.
_BF16_LAYOUT = [
    ('wout1', 128, 129), ('wout2_0', 128, 257), ('wout2_1', 128, 257),
    ('linw', 128, 256),
    ('wx1', 128, 12), ('wdt1', 8, 128),
    ('sel1_0', 12, 128), ('sel1_1', 12, 128),
    ('sel1_2', 12, 128), ('sel1_3', 12, 128),
    ('sel2_0', 20, 128), ('sel2_1', 20, 128),
    ('sel2_2', 20, 128), ('sel2_3', 20, 128),
    ('w2k0_0', 128, 256), ('w2k0_1', 128, 256),
    ('w2k1_0', 128, 256), ('w2k1_1', 128, 256),
    ('w2k2_0', 128, 256), ('w2k2_1', 128, 256),
    ('w2k3_0', 128, 256), ('w2k3_1', 128, 256),
    ('w2z_0', 128, 256), ('w2z_1', 128, 256),
    ('wx2_0', 128, 20), ('wx2_1', 128, 20),
    ('wdt2', 16, 256),
]


def _layout_offsets(layout):
    offs, c = {}, 0
    for name, rows, cols in layout:
        offs[name] = (c, rows, cols)
        c += cols
    return offs, c


def _pack_blob(layout, parts, np_dtype):
    offs, total = _layout_offsets(layout)
    blob = np.zeros((128, total), np_dtype)
    for name, (c0, rows, cols) in offs.items():
        p = np.asarray(parts[name])
        assert p.shape == (rows, cols), (name, p.shape, rows, cols)
        blob[:rows, c0:c0 + cols] = p
    return blob


def prep_weights(inputs, use_bf16):
    s1 = {k[3:]: inputs[k] for k in inputs if k.startswith('s1_')}
    s2 = {k[3:]: inputs[k] for k in inputs if k.startswith('s2_')}
    w1k, w1z, wx1, wdt1, wout1, cols1, sel1 = _prep_stage(s1, 128, 128, 8)
    w2k, w2z, wx2, wdt2, wout2, cols2, sel2 = _prep_stage(s2, 256, 256, 16)
    lnw2 = np.asarray(s2['lnw'], np.float32)
    lnb2 = np.asarray(s2['lnb'], np.float32)
    cols2 = np.concatenate([cols2, lnw2[:, None], lnb2[:, None]], 1)
    cols2 = np.ascontiguousarray(cols2, dtype=np.float32)

    bfdt = mybir.dt.np(BF16)
    lin_w = np.asarray(inputs['lin_w'], np.float32)
    lin_b = np.asarray(inputs['lin_b'], np.float32)
    lnw1 = np.asarray(s1['lnw'], np.float32)
    lnb1 = np.asarray(s1['lnb'], np.float32)
    linw = np.ascontiguousarray((lin_w * lnw1[None, :]).T)
    linb = (lin_w @ lnb1 + lin_b).astype(np.float32)

    pa = {'w1z': w1z, 'cols1': cols1,
          'eps': np.full((128, 1), LN_EPS, np.float32)}
    for k in range(4):
        pa[f'w1k{k}'] = w1k[k]
    pb = {'linb': np.stack([linb[:128], linb[128:]], 1)}
    for kt in range(2):
        pb[f'cols2_{kt}'] = cols2[kt * 128:(kt + 1) * 128]

    def with_mean_col(w, dout):
        return np.concatenate([w, w.sum(1, keepdims=True) / dout], 1)

    pc = {'wout1': with_mean_col(wout1, 128).astype(bfdt),
          'linw': linw.astype(bfdt),
          'wx1': wx1.astype(bfdt), 'wdt1': wdt1.astype(bfdt),
          'wdt2': wdt2.astype(bfdt)}
    for kt in range(2):
        pc[f'wout2_{kt}'] = with_mean_col(
            wout2[kt * 128:(kt + 1) * 128], 256).astype(bfdt)
    for k in range(4):
        pc[f'sel1_{k}'] = sel1[k][:12].astype(bfdt)
        pc[f'sel2_{k}'] = sel2[k][:20].astype(bfdt)
        for kt in range(2):
            pc[f'w2k{k}_{kt}'] = w2k[k][kt * 128:(kt + 1) * 128].astype(bfdt)
    for kt in range(2):
        pc[f'w2z_{kt}'] = w2z[kt * 128:(kt + 1) * 128].astype(bfdt)
        pc[f'wx2_{kt}'] = wx2[kt * 128:(kt + 1) * 128].astype(bfdt)
    return {
        'wfa': _pack_blob(_F32A_LAYOUT, pa, np.float32),
        'wfb': _pack_blob(_F32B_LAYOUT, pb, np.float32),
        'wbf': _pack_blob(_BF16_LAYOUT, pc, bfdt),
    }


# ----------------------------------------------------------------------------
# device program
# ----------------------------------------------------------------------------

F32R = mybir.dt.float32r

# engine assignment for movable ops: 'V' = DVE, 'P' = GPSIMD/Pool
ENG = {
    'm0': 'V',      # t_m0 = hs0 * C0
    'y1': 'V',      # t_y  = hs1 * C1
    'yadd': 'V',    # t_y += t_m0
    'yg': 'V',      # t_yg = t_y * silu(z)
    'tu': 'V',      # t_u = dt * xc
    'dbu0': 'V',    # dbu_0 = t_u * B_0
    'dbu1': 'V',    # dbu_1 = t_u * B_1
    'scan1': 'V',   # hs1 scan (dA1 recurrence)
    'da1': 'P',     # dA1 = dA0^2 (A1 == 2*A0) on Pool/DVE; 'A' = ACT Exp
    'ofa': 'P',     # output LN-affine per chunk
    'sttsplit': 1,  # y += dd*xc as 4x tensor_scalar + add instead of STT
    'repbcp': 'S',  # repB PSUM->SBUF copy (split ACT/DVE)
    'repccp': 'A',  # repC PSUM->SBUF copy
    'ypcp': 'A',    # yp PSUM->SBUF copy
    'xdblcp': 'A',  # xdbl PSUM->SBUF copy
    'mucp': 'V',    # mu column PSUM->SBUF copy (DVE or free-ACT)
}


def _copy(nc, cfg, eng, out, in_):
    if eng == 'V':
        nc.vector.tensor_copy(out, in_)
    elif eng == 'F':
        # Identity is present in every ACT table set, so these copies can
        # float outside the table-phase barrier groups and fill ACT gaps.
        nc.scalar.activation(out, in_, AF.Identity)
    elif eng == 'S':
        # split halves across ACT and DVE so they run in parallel
        h = out.shape[-1] // 2
        nc.scalar.activation(out[:, :h], in_[:, :h], AF.Identity)
        nc.vector.tensor_copy(out[:, h:], in_[:, h:])
    else:
        cfg['act'](out, in_, AF.Identity)


if os.environ.get("KB_NOPOOL"):
    for _k, _v in list(ENG.items()):
        if _v == 'P':
            ENG[_k] = 'V'


def _veng(nc, eng):
    return nc.vector if eng == 'V' else nc.gpsimd


def _tile(pool, shape, dtype, tag, bufs=None):
    return pool.tile(shape, dtype, tag=tag, name=tag, bufs=bufs)


class _View:
    """2D window into a packed blob tile; slicing composes with the window
    offset and optionally bitcasts the resulting AP."""

    def __init__(self, tile, r0, c0, rows, cols, bc=None):
        self.t, self.r0, self.c0 = tile, r0, c0
        self.rows, self.cols, self.bc = rows, cols, bc

    def __getitem__(self, idx):
        if not isinstance(idx, tuple):
            idx = (idx, slice(None))
        rs, cs = idx
        r = rs.indices(self.rows)
        c = cs.indices(self.cols)
        ap = self.t[self.r0 + r[0]: self.r0 + r[1],
                    self.c0 + c[0]: self.c0 + c[1]]
        return ap.bitcast(self.bc) if self.bc is not None else ap


def _mmr(nc, out, lhsT, rhs, **kw):
    """Matmul helper: bf16 operands go straight through; fp32 operands are
    bitcast to float32r (single-pass on the PE vs two half-speed passes)."""
    if lhsT.dtype == BF16:
        nc.tensor.matmul(out, lhsT, rhs, **kw)
    else:
        nc.tensor.matmul(out, lhsT.bitcast(F32R), rhs.bitcast(F32R), **kw)


class _ActChain:
    """Groups ACT instructions into table-set phases separated by no-op
    barrier instructions, so the scheduler can reorder freely within a phase
    (same table set) but cannot interleave phases (which would make bacc
    insert a ~2.7us ACT table load per out-of-phase function switch)."""

    def __init__(self, nc, bar_tile):
        self.nc = nc
        self.bar_tile = bar_tile
        self.group = []
        self.barrier = None

    def new_group(self):
        from concourse.tile_rust import add_dep_helper
        if not self.group:
            return
        bar = self.nc.scalar.activation(self.bar_tile[:], self.bar_tile[:],
                                        AF.Identity)
        barc = bar.ins if hasattr(bar, 'ins') else bar
        for op in self.group:
            add_dep_helper(barc, op, sync=False, reason="act phase barrier")
        self.barrier = barc
        self.group = []

    def __call__(self, *args, **kwargs):
        from concourse.tile_rust import add_dep_helper
        inst = self.nc.scalar.activation(*args, **kwargs)
        cur = inst.ins if hasattr(inst, 'ins') else inst
        if self.barrier is not None:
            add_dep_helper(cur, self.barrier, sync=False,
                           reason="act phase order")
        self.group.append(cur)
        return inst


def _stage_phase_a(nc, pools, cfg, s0):
    """in_proj (conv-folded) + z + SiLU for one span -> xc/sz span planes."""
    ps_mm = pools['mm']
    P_in, P = cfg['P_in'], cfg['P']
    planes, wk, wz, cols = (cfg['in_planes'], cfg['wk_sb'], cfg['wz_sb'],
                            cfg['cols_sb'])
    xc_sp, sz_sp = cfg['xc_sp'], cfg['sz_sp']
    for ci in range(SPAN // CH):
        c0 = s0 + ci * CH
        lc = ci * CH
        for mi in range(P):
            ms = slice(mi * 128, (mi + 1) * 128)
            xc_ps = _tile(ps_mm, [128, CH], F32, "mm", 4)
            nmm = 4 * P_in
            i = 0
            for k in range(4):
                for kt in range(P_in):
                    _mmr(nc, xc_ps[:], wk[k][kt][:, ms],
                         planes[kt][:, c0 + k: c0 + k + CH],
                         start=(i == 0), stop=(i == nmm - 1))
                    i += 1
            if c0 == 0:
                nc.vector.tensor_add(xc_ps[:, 0:3], xc_ps[:, 0:3],
                                     cols[mi][:, 6:9])
            cfg['act'](xc_sp[mi][:, lc:lc + CH], xc_ps[:], AF.Silu,
                                 bias=cols[mi][:, 0:1])

            z_ps = _tile(ps_mm, [128, CH], F32, "mm", 4)
            for kt in range(P_in):
                _mmr(nc, z_ps[:], wz[kt][:, ms],
                     planes[kt][:, c0 + 3: c0 + 3 + CH],
                     start=(kt == 0), stop=(kt == P_in - 1))
            cfg['act'](sz_sp[mi][:, lc:lc + CH], z_ps[:], AF.Silu,
                                 bias=cols[mi][:, 1:2])


def _stage_phase_b(nc, pools, cfg, s0, hs_prev):
    """Everything after SiLU for one span (natural_log_exp table set only)."""
    sb = pools['sb']
    ps_mm, ps_rep, ps_o = pools['mm'], pools['rep'], pools['o']
    P_in, P, r, dout = cfg['P_in'], cfg['P'], cfg['r'], cfg['dout']
    rw = r + 4
    wxs, wdts, wouts, cols = (cfg['wx_sb'], cfg['wdt_sb'], cfg['wout_sb'],
                              cfg['cols_sb'])
    xc_sp, sz_sp = cfg['xc_sp'], cfg['sz_sp']

    ddxc_sp = []
    for mi in range(P):
        t_dx = _tile(pools['spans'], [128, SPAN], BF16, f"ddxc{mi}",
                     1 if SPAN >= cfg['L'] else 2)
        nc.vector.tensor_scalar(t_dx[:], xc_sp[mi][:, :],
                                cfg['cols_sb'][mi][:, 5:6], None,
                                ALU.mult, ALU.bypass)
        ddxc_sp.append(t_dx)
    cfg['ddxc_sp'] = ddxc_sp

    for ci in range(SPAN // CH):
        c0 = s0 + ci * CH
        lc = ci * CH
        lcs = slice(lc, lc + CH)
        # ---- wx projection -> dtin rows + B/C rows ----
        xdbl_ps = _tile(ps_mm, [128, CH], F32, "mm", 4)
        for kt in range(P_in):
            _mmr(nc, xdbl_ps[:rw, :], wxs[kt][:], xc_sp[kt][:, lcs],
                 start=(kt == 0), stop=(kt == P_in - 1))
        xdbl = _tile(sb, [rw, CH], BF16, "xdbl", 2)
        _copy(nc, cfg, ENG['xdblcp'], xdbl[:], xdbl_ps[:rw, :])

        # ---- dt = softplus = Ln(Exp(raw + bdt) + 1); dA_n = exp(A_n*dt) ----
        dt_sb = []
        dA_sb = [[None] * P, [None] * P]
        if P == 2 and cfg['sp_const']:
            # bdt and A0 match between the two channel halves, so the whole
            # chain runs as single [128, 2*CH] ops (halves = mi blocks).
            dt_ps = _tile(ps_rep, [128, 2 * CH], F32, "rep", 1)
            for mi in range(P):
                _mmr(nc, dt_ps[:, mi * CH:(mi + 1) * CH],
                     wdts[:, mi * 128:(mi + 1) * 128], xdbl[:r, :])
            t_e = _tile(sb, [128, 2 * CH], F32, "te2", 2)
            cfg['act'](t_e[:], dt_ps[:], AF.Exp, bias=cols[0][:, 2:3])
            t_dt = _tile(sb, [128, 2 * CH], BF16, "dt2", 2)
            cfg['act'](t_dt[:], t_e[:], AF.Ln, bias=1.0)
            t_dA0 = _tile(sb, [128, 2 * CH], F32, "dA0w", 2)
            cfg['act'](t_dA0[:], t_dt[:], AF.Exp, scale=cols[0][:, 3:4])
            t_dA1 = _tile(sb, [128, 2 * CH], F32, "dA1w", 2)
            if cfg['a2_is_2a1'] and ENG['da1'] != 'A':
                _veng(nc, ENG['da1']).tensor_mul(t_dA1[:], t_dA0[:], t_dA0[:])
            else:
                cfg['act'](t_dA1[:], t_dt[:], AF.Exp, scale=cols[0][:, 4:5])
            for mi in range(P):
                mcs = slice(mi * CH, (mi + 1) * CH)
                dt_sb.append(_View(t_dt, 0, mi * CH, 128, CH))
                dA_sb[0][mi] = _View(t_dA0, 0, mi * CH, 128, CH)
                dA_sb[1][mi] = _View(t_dA1, 0, mi * CH, 128, CH)
        else:
            for mi in range(P):
                ms = slice(mi * 128, (mi + 1) * 128)
                dt_ps = _tile(ps_mm, [128, CH], F32, "mm", 4)
                _mmr(nc, dt_ps[:], wdts[:, ms], xdbl[:r, :])
                t_e = _tile(sb, [128, CH], F32, "scr1", 3)
                cfg['act'](t_e[:], dt_ps[:], AF.Exp,
                                     bias=cols[mi][:, 2:3])
                t_dt = _tile(sb, [128, CH], BF16, "dt", 3)
                cfg['act'](t_dt[:], t_e[:], AF.Ln, bias=1.0)
                dt_sb.append(t_dt)
                t_dA0 = _tile(sb, [128, CH], F32, "dA0", 3)
                cfg['act'](t_dA0[:], t_dt[:], AF.Exp,
                           scale=cols[mi][:, 3:4])
                dA_sb[0][mi] = t_dA0
                t_dA1 = _tile(sb, [128, CH], F32, "dA1", 3)
                if cfg['a2_is_2a1'] and ENG['da1'] != 'A':
                    _veng(nc, ENG['da1']).tensor_mul(t_dA1[:], t_dA0[:],
                                                     t_dA0[:])
                else:
                    cfg['act'](t_dA1[:], t_dt[:], AF.Exp,
                               scale=cols[mi][:, 4:5])
                dA_sb[1][mi] = t_dA1

        # ---- replicate B rows across partitions (PE sel-matmuls) ----
        repB_ps = _tile(ps_rep, [128, 2 * CH], F32, "rep", 1)
        for j in range(2):
            _mmr(nc, repB_ps[:, j * CH:(j + 1) * CH],
                 cfg['sel_sb'][j][:], xdbl[:])
        repB = _tile(sb, [128, 2 * CH], BF16, "repBsb", 2)
        _copy(nc, cfg, ENG['repbcp'], repB[:], repB_ps[:])
        hs2 = [None] * P
        for mi in range(P):
            t_u = _tile(sb, [128, CH], BF16, "scr1", 3)
            _veng(nc, ENG['tu']).tensor_mul(
                t_u[:], dt_sb[mi][:, :], xc_sp[mi][:, lcs])
            t_hs2 = _tile(sb, [128, 2 * CH], BF16, "hs2", 3)
            hs2[mi] = t_hs2
            for n in range(2):
                t_dbu = _tile(sb, [128, CH], BF16, f"dbu{n}", 2)
                _veng(nc, ENG[f'dbu{n}']).tensor_mul(
                    t_dbu[:], t_u[:], repB[:, n * CH:(n + 1) * CH])
                init = 0.0 if c0 == 0 else hs_prev[n][mi][:, CH - 1:CH]
                seng = nc.vector if (n == 0 or ENG['scan1'] == 'V') \
                    else nc.gpsimd
                seng.tensor_tensor_scan(t_hs2[:, n * CH:(n + 1) * CH],
                                        dA_sb[n][mi][:, :],
                                        t_dbu[:], init, ALU.mult, ALU.add)
                hs_prev[n][mi] = _View(t_hs2, 0, n * CH, 128, CH)

        repC_ps = _tile(ps_rep, [128, 2 * CH], F32, "rep", 1)
        for j in range(2):
            _mmr(nc, repC_ps[:, j * CH:(j + 1) * CH],
                 cfg['sel_sb'][2 + j][:], xdbl[:])
        repC = _tile(sb, [128, 2 * CH], BF16, "repCsb", 2)
        _copy(nc, cfg, ENG['repccp'], repC[:], repC_ps[:])

        # ---- y = (hs0*C0 + hs1*C1 + dd*xc) * silu(z) ----
        yg_sb = []
        for mi in range(P):
            t_m = _tile(sb, [128, 2 * CH], BF16, "ym", 2)
            _veng(nc, ENG['m0']).tensor_mul(t_m[:], hs2[mi][:], repC[:])
            t_y = _tile(sb, [128, CH], BF16, "y", 2)
            _veng(nc, ENG['yadd']).tensor_add(t_y[:], t_m[:, 0:CH],
                                              t_m[:, CH:2 * CH])
            nc.vector.tensor_add(t_y[:], t_y[:], cfg['ddxc_sp'][mi][:, lcs])
            t_yg = _tile(sb, [128, CH], BF16, "yg", 3)
            _veng(nc, ENG['yg']).tensor_mul(t_yg[:], t_y[:], sz_sp[mi][:, lcs])
            yg_sb.append(t_yg)

        # ---- out_proj (time-major, extra weight column = rowmean for mu)
        #      + LayerNorm: ssq via DVE tensor_tensor_reduce off PSUM ----
        nsub = CH // SUB
        var = _tile(sb, [SUB, nsub], F32, "var")
        mu = _tile(sb, [SUB, nsub], F32, "mu")
        yp_tiles = []
        for g in range(nsub):
            cs = slice(g * SUB, (g + 1) * SUB)
            yp_ps = _tile(ps_o, [SUB, dout + 1], F32, "yp", 2)
            for mi in range(P):
                nc.tensor.matmul(yp_ps[:], yg_sb[mi][:, cs], wouts[mi][:],
                                 start=(mi == 0), stop=(mi == P - 1))
            if ENG['mucp'] == 'V':
                nc.vector.tensor_copy(mu[:, g:g + 1],
                                      yp_ps[:, dout:dout + 1])
            else:
                nc.scalar.activation(mu[:, g:g + 1],
                                     yp_ps[:, dout:dout + 1], AF.Identity)
            yp = _tile(sb, [SUB, dout], BF16, "ypsb", 4)
            _copy(nc, cfg, ENG['ypcp'], yp[:], yp_ps[:, :dout])
            # per-timestep variance via native bn_stats/bn_aggr
            st6 = _tile(sb, [SUB, 6], F32, "st6", 2)
            if os.environ.get("KB_BNPSUM"):
                nc.vector.bn_stats(st6[:], yp_ps[:, :dout])
            else:
                nc.vector.bn_stats(st6[:], yp[:])
            st2 = _tile(sb, [SUB, 2], F32, "st2", 2)
            nc.vector.bn_aggr(st2[:], st6[:])
            nc.vector.tensor_copy(var[:, g:g + 1], st2[:, 1:2])
            yp_tiles.append(yp)
        lnv = _tile(sb, [SUB, nsub], F32, "lnv")
        cfg['act'](lnv[:], var[:], AF.Ln, bias=cfg['eps'])
        rstd = _tile(sb, [SUB, nsub], F32, "rstd")
        cfg['act'](rstd[:], lnv[:], AF.Exp, scale=-0.5)
        for g in range(nsub):
            tn = _tile(sb, [SUB, dout], BF16, "tn", 4)
            nc.vector.tensor_scalar(tn[:], yp_tiles[g][:], mu[:, g:g + 1],
                                    rstd[:, g:g + 1], ALU.subtract, ALU.mult)
            cfg['emit'](tn, c0, g)
        if 'emit_flush' in cfg:
            cfg['emit_flush'](c0)


def build_program(L=4096, use_bf16=False, a2_is_2a1=False,
                  sp_const=True):
    nc = bacc.Bacc()
    dp = nc.declare_dram_parameter
    offs_a, tot_a = _layout_offsets(_F32A_LAYOUT)
    offs_b, tot_b = _layout_offsets(_F32B_LAYOUT)
    offs_c, tot_c = _layout_offsets(_BF16_LAYOUT)
    x_d = dp("x", [128, L], F32R, isOutput=False)
    wfa_d = dp("wfa", [128, tot_a], F32R, isOutput=False)
    wfb_d = dp("wfb", [128, tot_b], F32R, isOutput=False)
    wbf_d = dp("wbf", [128, tot_c], BF16, isOutput=False)
    out_d = dp("out", [256, L], F32, isOutput=True)

    dma = nc.sync.dma_start

    with tile.TileContext(nc) as tc, ExitStack() as ctx:
        consts = ctx.enter_context(tc.tile_pool(name="consts", bufs=1))
        planes = ctx.enter_context(tc.tile_pool(name="planes", bufs=1))
        spans = ctx.enter_context(tc.tile_pool(name="spans", bufs=1))
        sb = ctx.enter_context(tc.tile_pool(name="sb", bufs=2))
        ps_mm = ctx.enter_context(
            tc.tile_pool(name="psmm", bufs=2, space=bass.MemorySpace.PSUM))
        ps_rep = ctx.enter_context(
            tc.tile_pool(name="psrep", bufs=1, space=bass.MemorySpace.PSUM))
        ps_o = ctx.enter_context(
            tc.tile_pool(name="pso", bufs=1, space=bass.MemorySpace.PSUM))
        pools = {'sb': sb, 'mm': ps_mm, 'rep': ps_rep, 'o': ps_o,
                 'spans': spans}

        # x first so its (large) transfer overlaps the weight-blob DMAs
        # instead of queueing behind them.
        xpad = planes.tile([128, L + 3], F32R, tag="xpad", name="xpad")
        nc.gpsimd.memset(xpad[:, 0:3].bitcast(F32), 0.0)
        wfa_sb = consts.tile([128, tot_a], F32R, tag="wfa", name="wfa")
        dma(wfa_sb[:], wfa_d[:])
        xq = L // 4
        for q in range(4):
            dma(xpad[:, 3 + q * xq: 3 + (q + 1) * xq],
                x_d[:, q * xq:(q + 1) * xq])
        wfb_sb = consts.tile([128, tot_b], F32R, tag="wfb", name="wfb")
        dma(wfb_sb[:], wfb_d[:])
        wbf_sb = consts.tile([128, tot_c], BF16, tag="wbf", name="wbf")
        dma(wbf_sb[:], wbf_d[:])

        def va(name, bc=None):
            c0, rows, cols = offs_a[name]
            return _View(wfa_sb, 0, c0, rows, cols, bc)

        def vb(name, bc=None):
            c0, rows, cols = offs_b[name]
            return _View(wfb_sb, 0, c0, rows, cols, bc)

        def vc(name):
            c0, rows, cols = offs_c[name]
            return _View(wbf_sb, 0, c0, rows, cols)

        w1k_sb = [[va(f'w1k{k}')] for k in range(4)]
        w1z_sb = [va('w1z')]
        wx1_sb = [vc('wx1')]
        wdt1_sb = vc('wdt1')
        wout1_sb = [vc('wout1')]
        cols1_sb = [va('cols1', F32)]
        w2k_sb = [[vc(f'w2k{k}_{kt}') for kt in range(2)] for k in range(4)]
        w2z_sb = [vc(f'w2z_{kt}') for kt in range(2)]
        wx2_sb = [vc(f'wx2_{kt}') for kt in range(2)]
        wdt2_sb = vc('wdt2')
        wout2_sb = [vc(f'wout2_{kt}') for kt in range(2)]
        cols2_sb = [vb(f'cols2_{kt}', F32) for kt in range(2)]
        linw_sb = vc('linw')
        sel1_sb = [vc(f'sel1_{j}') for j in range(4)]
        sel2_sb = [vc(f'sel2_{j}') for j in range(4)]
        linb_sb = [_View(wfb_sb, 0, offs_b['linb'][0] + kt, 128, 1, F32)
                   for kt in range(2)]

        bar_tile = consts.tile([1, 1], F32, tag="actbar", name="actbar")
        nc.gpsimd.memset(bar_tile[:], 0.0)
        act_chain = _ActChain(nc, bar_tile)
        t1n = planes.tile([128, L], BF16, tag="t1n", name="t1n")
        t2pad = [planes.tile([128, L + 3], BF16, tag=f"t2pad{mi}",
                             name=f"t2pad{mi}") for mi in range(2)]
        for mi in range(2):
            nc.gpsimd.memset(t2pad[mi][:, 0:3], 0.0)

        def span_tiles(P):
            nb = 1 if SPAN >= L else 2
            xc_sp = [_tile(spans, [128, SPAN], BF16, f"xcsp{mi}", nb)
                     for mi in range(P)]
            sz_sp = [_tile(spans, [128, SPAN], BF16, f"szsp{mi}", nb)
                     for mi in range(P)]
            return xc_sp, sz_sp

        # ---- stage 1 (LN output lands bf16, DMA-transposed into t1n) ----
        def emit1(tn, c0, g):
            if os.environ.get("KB_NOXPOSE"):
                # bisect mode: wrong data, right shapes (no transpose)
                nc.vector.tensor_copy(
                    t1n[:, c0 + g * SUB: c0 + (g + 1) * SUB], tn[:])
            else:
                nc.sync.dma_start_transpose(
                    t1n[:, c0 + g * SUB: c0 + (g + 1) * SUB], tn[:])

        cfg1 = dict(
            L=L, P_in=1, P=1, r=8, dout=128, in_planes=[xpad],
            wk_sb=w1k_sb, wz_sb=w1z_sb, wx_sb=wx1_sb, wdt_sb=wdt1_sb,
            wout_sb=wout1_sb, cols_sb=cols1_sb, sel_sb=sel1_sb,
            sp_const=False,
            eps=va('eps', F32)[:, 0:1], emit=emit1, act=act_chain,
            a2_is_2a1=a2_is_2a1)
        hs_prev1 = [[None], [None]]

        # ---- stage 2 (the linear+SiLU joins each span's SiLU phase) ----
        # LN output (bf16, time-major) is DMA-transposed into per-half
        # planes; the LN affine runs per chunk on DVE, then one output DMA
        # per (half, chunk).
        o_sp = [planes.tile([128, L], BF16, tag=f"osp{ct}", name=f"osp{ct}")
                for ct in range(2)]

        def emit2(tn, c0, g):
            for ct in range(2):
                if os.environ.get("KB_NOXPOSE"):
                    nc.vector.tensor_copy(
                        o_sp[ct][:, c0 + g * SUB: c0 + (g + 1) * SUB],
                        tn[:, ct * 128:(ct + 1) * 128])
                else:
                    nc.sync.dma_start_transpose(
                        o_sp[ct][:, c0 + g * SUB: c0 + (g + 1) * SUB],
                        tn[:, ct * 128:(ct + 1) * 128])

        def emit2_flush(c0):
            for ct in range(2):
                of = _tile(sb, [128, CH], F32, "ofc")
                if ENG['ofa'] == 'A':
                    nc.scalar.activation(of[:], o_sp[ct][:, c0:c0 + CH],
                                         AF.Identity,
                                         scale=cols2_sb[ct][:, 9:10],
                                         bias=cols2_sb[ct][:, 10:11])
                else:
                    _veng(nc, ENG['ofa']).tensor_scalar(
                        of[:], o_sp[ct][:, c0:c0 + CH],
                        cols2_sb[ct][:, 9:10], cols2_sb[ct][:, 10:11],
                        ALU.mult, ALU.add)
                # SWDGE (Pool-issued DMA): keeps HWDGE free for the
                # LN-output transposes.
                if os.environ.get("KB_NOSWDGE"):
                    dma(out_d[ct * 128:(ct + 1) * 128, c0:c0 + CH], of[:])
                else:
                    nc.gpsimd.dma_start(
                        out_d[ct * 128:(ct + 1) * 128, c0:c0 + CH], of[:])

        cfg2 = dict(
            L=L, P_in=2, P=2, r=16, dout=256, in_planes=t2pad,
            wk_sb=w2k_sb, wz_sb=w2z_sb, wx_sb=wx2_sb, wdt_sb=wdt2_sb,
            wout_sb=wout2_sb, cols_sb=cols2_sb, sel_sb=sel2_sb,
            sp_const=sp_const,
            eps=va('eps', F32)[:, 0:1], emit=emit2, emit_flush=emit2_flush,
            act=act_chain, a2_is_2a1=a2_is_2a1)
        hs_prev2 = [[None, None], [None, None]]

        def s1_a(s0):
            xc_sp, sz_sp = span_tiles(1)
            cfg1['xc_sp'], cfg1['sz_sp'] = xc_sp, sz_sp
            cfg1[f'sp{s0}'] = (xc_sp, sz_sp)
            _stage_phase_a(nc, pools, cfg1, s0)

        def s1_b(s0):
            cfg1['xc_sp'], cfg1['sz_sp'] = cfg1[f'sp{s0}']
            _stage_phase_b(nc, pools, cfg1, s0, hs_prev1)

        def s2_lin_a(s0):
            # linear + silu for this span (same SiLU table set as phase A)
            for c0 in range(s0, s0 + SPAN, CH):
                for mi in range(2):
                    ms = slice(mi * 128, (mi + 1) * 128)
                    lp = _tile(ps_mm, [128, CH], F32, "mm", 4)
                    nc.tensor.matmul(lp[:], linw_sb[:, ms],
                                     t1n[:, c0:c0 + CH])
                    act_chain(t2pad[mi][:, 3 + c0: 3 + c0 + CH],
                              lp[:], AF.Silu,
                              bias=linb_sb[mi][:, 0:1])
            xc_sp, sz_sp = span_tiles(2)
            cfg2['xc_sp'], cfg2['sz_sp'] = xc_sp, sz_sp
            cfg2[f'sp{s0}'] = (xc_sp, sz_sp)
            _stage_phase_a(nc, pools, cfg2, s0)

        def s2_b(s0):
            cfg2['xc_sp'], cfg2['sz_sp'] = cfg2[f'sp{s0}']
            _stage_phase_b(nc, pools, cfg2, s0, hs_prev2)

        # Software-pipelined phase schedule: stage-2 work on span k shares
        # table-set groups with stage-1 work on span k+1, removing the
        # stage-boundary bubble.  (silu groups: s1A / s1A+lin+s2A / ...;
        # lnexp groups: s1B / s1B+s2B / ...)
        spans_l = list(range(0, L, SPAN))
        sched = []
        for i, s0 in enumerate(spans_l):
            ga = [('1a', s0)]
            gb = [('1b', s0)]
            if i > 0:
                ga.append(('2a', spans_l[i - 1]))
                gb.append(('2b', spans_l[i - 1]))
            sched.append(ga)
            sched.append(gb)
        sched.append([('2a', spans_l[-1])])
        sched.append([('2b', spans_l[-1])])
        fns = {'1a': s1_a, '1b': s1_b, '2a': s2_lin_a, '2b': s2_b}
        for group in sched:
            act_chain.new_group()
            for tag, s0 in group:
                fns[tag](s0)

    nc.finalize()
    return nc


# ----------------------------------------------------------------------------
# entry point
# ----------------------------------------------------------------------------

_NC = {}


def kernel(**inputs):
    global last_exec_time_ns
    use_bf16 = os.environ.get("KBENCH_BF16", "0") == "1"
    inputs = {k: np.asarray(v) for k, v in inputs.items()}
    weights = prep_weights(inputs, use_bf16)
    x = inputs['x'].astype(np.float32)          # [8, 128, 64, 64]
    b, c, h, w = x.shape
    L = h * w

    a1 = -np.exp(np.asarray(inputs['s1_alog'], np.float32))
    a2 = -np.exp(np.asarray(inputs['s2_alog'], np.float32))
    a2_is_2a1 = (np.allclose(a1[:, 1], 2 * a1[:, 0], rtol=1e-6) and
                 np.allclose(a2[:, 1], 2 * a2[:, 0], rtol=1e-6))
    bdt2 = np.asarray(inputs['s2_bdt'], np.float32)
    sp_const = (np.allclose(bdt2[:128], bdt2[128:]) and
                np.allclose(a2[:128, 0], a2[128:, 0], rtol=1e-6))
    key = (L, use_bf16, a2_is_2a1, sp_const)
    if key not in _NC:
        _NC[key] = build_program(L, use_bf16, a2_is_2a1, sp_const)

    in_maps = [dict(weights, x=np.ascontiguousarray(x[i].reshape(c, L)))
               for i in range(NCORES)]
    res = run_bass_kernel_spmd(
        _NC[key], in_maps, list(range(NCORES)),
        trace=bool(os.environ.get("KBENCH_TRACE")))
    last_exec_time_ns = res.exec_time_ns
    out = np.stack([np.asarray(res.results[i]['out'], np.float32)
                    .reshape(256, h, w) for i in range(NCORES)])
    return out

